# revision 10
# baseline (speedup 1.0000x reference)
"""Self-contained Trainium2 Bass kernel for the 2-layer GAT problem.

Accepts FULL inputs (as produced by setup_inputs()), shards across the
8 NeuronCores internally, returns the full [100000, 1] float32 output.

v2 design (gather-minimized): per core, edges (self-loops excluded —
handled analytically) are laid out in 128 rows = 8 src-chunk bands x 16
fixed dst-range pieces (784 dsts each), each row's runs (per-dst edge
groups) prefixed with a dummy slot whose gather index points into an
exp(v)-table section, so ONE ap_gather pass per layer fetches both
x/P[src] (real slots) and exp(v_dst) (dummy slots).  A copy-forward
scan spreads the per-run dst factor; numer = max(w, w^0.2) realizes
exp(LeakyRelu(u+v)) = f(e^u * e^v).  Segmented scans accumulate run
sums, written d-interleaved so one d-merged ap_gather per layer
extracts all channels' run-end values; 0/1 selection-vector matmuls
(valid row is static: 16*band + dst//784) reduce across bands.
"""
import numpy as np
import ml_dtypes

N_NODES = 100000
NC = 8
NSH = 12500           # nodes per shard/chunk
PIECE = 784           # dsts per row piece
NSHP = 12544          # 128 * 98
W98 = 98
XW = 12512            # x-part width in gather table
ZCOL = XW + XW        # reserved zero column (start)
TABW = ZCOL + 16      # table width (25040)

# ===================== tile drain workaround =====================
"""This walrus build allows at most 1 sync-wait on SP CTRL instructions,
but TileContext's tail drain aggregates all end-of-kernel waits onto one
Drain.  Split them across nops instead."""
import concourse.tile as tile
from concourse import mybir
from bass_rust import ScopedClock


def _patched_drain_and_barrier(self, tick_clock, wait_clock):
    nc = self.nc
    probe = nc.sync.nop()
    wait_clock.add_sem_waits(probe.ins, ScopedClock({None: tick_clock.global_clock}))
    si = probe.ins.sync_info
    waits = list(si.on_wait) if si is not None else []
    if si is not None:
        si.on_wait = waits[:1]
    for w in waits[1:]:
        nop = nc.sync.nop()
        nop.ins.sync_info = mybir.SyncInfo(on_wait=[w], on_update=[])
    nc.sync.drain()
    nc.all_engine_barrier()
    popped = nc._tile_sem_poison_stack.pop()
    assert popped is self._sem_poison
    nc.clear_and_free_semaphores(list(self.sems.allocated().values()))
    nc.all_engine_barrier()


def install():
    tile.TileContext._drain_and_barrier = _patched_drain_and_barrier


# ===================== host preprocessing =====================

def prep(edge_index: np.ndarray, N: int):
    """Layout per core: rows = 16*src_chunk + dst_local//784, slots =
    [dummy][edges...] per (row, dst)-run, sorted by dst within row."""
    assert N == N_NODES
    src = np.asarray(edge_index[0], np.int64)
    dst = np.asarray(edge_index[1], np.int64)
    core = dst // NSH

    cores = []
    maxslots = 0
    for c in range(NC):
        sel = core == c
        s = src[sel]
        dl = (dst[sel] - c * NSH).astype(np.int64)
        band = s // NSH
        sl = (s % NSH).astype(np.int64)
        row = band * 16 + dl // PIECE
        order = np.lexsort((dl, row))
        rs = row[order].astype(np.int64)
        ds = dl[order]
        sls = sl[order]
        n = len(rs)
        newrun = np.empty(n, dtype=bool)
        newrun[0] = True
        newrun[1:] = (rs[1:] != rs[:-1]) | (ds[1:] != ds[:-1])
        run_first = np.flatnonzero(newrun)
        run_id = np.cumsum(newrun) - 1
        run_row = rs[run_first]
        row_start = np.searchsorted(rs, np.arange(128))
        within = np.arange(n) - row_start[rs]
        run_ord = np.arange(len(run_first)) - np.searchsorted(run_row, run_row)
        slot = within + run_ord[run_id] + 1
        dummy_slot = slot[run_first] - 1
        row_slots = np.bincount(rs, minlength=128) + np.bincount(run_row, minlength=128)
        maxslots = max(maxslots, int(row_slots.max()))
        cores.append(dict(rs=rs, ds=ds, sls=sls, slot=slot, run_first=run_first,
                          run_row=run_row, dummy_slot=dummy_slot))

    L = ((maxslots + 1 + 31) // 32) * 32   # reserved col at L-1; L/8 % 4 == 0
    LW = L // 16

    for c in range(NC):
        d = cores[c]
        rs, ds, sls, slot = d["rs"], d["ds"], d["sls"], d["slot"]
        run_first, run_row, dummy_slot = d["run_first"], d["run_row"], d["dummy_slot"]
        offs = np.full((128, L), ZCOL, dtype=np.int16)
        dmask = np.zeros((128, L), dtype=np.float32)
        kill = np.zeros((128, L), dtype=np.float32)
        offs[rs, slot] = sls.astype(np.int16)
        kill[rs, slot] = 1.0
        run_dst = ds[run_first]
        offs[run_row, dummy_slot] = (XW + run_dst).astype(np.int16)
        dmask[run_row, dummy_slot] = 1.0
        offs[:, L - 1] = ZCOL
        dmask[:, L - 1] = 1.0

        # run ends: last edge of run r is run_first[r+1]-1 (or n-1)
        n = len(rs)
        run_last = np.append(run_first[1:], n) - 1
        end_slot = slot[run_last]
        ends = np.full((8, NSHP), L - 1, dtype=np.int64)
        ends[run_row // 16, run_dst] = end_slot
        ends_w = np.zeros((128, NSHP // 16), dtype=np.int16)
        for b in range(8):
            ends_w[16 * b:16 * b + 16, :] = ends[b].reshape(-1, 16).T.astype(np.int16)

        idxw = np.zeros((16, 128, LW), dtype=np.int16)
        offs3 = offs.reshape(8, 16, L)
        for b in range(8):
            for p in range(16):
                idxw[p, 16 * b:16 * b + 16, :] = offs3[b, p].reshape(LW, 16).T

        selm = np.zeros((128, 16), dtype=np.float32)
        selm[np.arange(128), np.arange(128) % 16] = 1.0

        cores[c] = dict(
            idxw=idxw.reshape(16 * 128, LW),
            dmask=dmask.astype(ml_dtypes.bfloat16),
            kill=kill.astype(ml_dtypes.bfloat16),
            ends_w=ends_w, selm=selm,
        )

    return dict(N=N, L=L, LW=LW, cores=cores)


# ===================== bass kernel builder =====================
from contextlib import ExitStack
import concourse.bass as bass
import concourse.bacc as bacc
from concourse import library_config
install()

F32 = mybir.dt.float32
I16 = mybir.dt.int16
BF16 = mybir.dt.bfloat16
AF = mybir.ActivationFunctionType
ALU = mybir.AluOpType
AX = mybir.AxisListType


def build(pp, dbg=False):
    L, LW = pp["L"], pp["LW"]
    NQ = 8
    LQ = L // NQ
    assert L % 16 == 0 and LQ % 4 == 0

    nc = bacc.Bacc("TRN2", target_bir_lowering=False, debug=False,
                   num_devices=8)

    def din(name, shape, dt=F32):
        return nc.dram_tensor(name, shape, dt, kind="ExternalInput")

    xc = din("xc", [1, NC * NSHP])          # x in chunked-12544 layout
    xsh = din("xsh", [1, NSHP])             # this core's shard of x
    W1 = din("W1", [1, 20])
    a_src1 = din("a_src1", [1, 20])
    a_dst1 = din("a_dst1", [1, 20])
    W2T = din("W2T", [1, 400])
    a_src2 = din("a_src2", [1, 20])
    a_dst2 = din("a_dst2", [1, 20])
    b2 = din("b2", [1, 20])
    Wl = din("Wl", [1, 20])
    bl = din("bl", [1, 1])
    idxw_d = din("idxw", [16 * 128, LW], I16)
    dmask_d = din("dmask", [128, L], BF16)
    kill_d = din("kill", [128, L], BF16)
    ends_d = din("ends_w", [128, NSHP // 16], I16)
    selm_d = din("selm", [128, 16])

    y_out = nc.dram_tensor("y", [1, NSHP], F32, kind="ExternalOutput")
    dbg_names = (["den1", "P1", "Pn", "den2", "Rp", "Rm", "b2nd", "s20",
                  "s21", "pf"] if dbg else [])
    dbg_outs = {nm: nc.dram_tensor("dbg_" + nm, [1, NSHP], F32,
                                   kind="ExternalOutput")
                for nm in dbg_names}

    b1_dram = nc.dram_tensor("b1d", [1, NSHP], F32)
    b2_dram = nc.dram_tensor("b2d", [1, NSHP], F32)
    p_local = nc.dram_tensor("p_local", [1, NSHP], F32)
    p_full = nc.dram_tensor("p_full", [1, NC * NSHP], F32, addr_space="Shared")
    S1 = nc.dram_tensor("S1", [1, NSHP * 2], F32)
    S2 = nc.dram_tensor("S2", [1, NSHP * 3], F32)

    with tile.TileContext(nc) as tc, ExitStack() as ctx:
        consts = ctx.enter_context(tc.tile_pool(name="consts", bufs=1))
        smalls = ctx.enter_context(tc.tile_pool(name="smalls", bufs=2))
        nodep = ctx.enter_context(tc.tile_pool(name="node", bufs=1))
        bigp = ctx.enter_context(tc.tile_pool(name="big", bufs=1))
        gxp = ctx.enter_context(tc.tile_pool(name="gx", bufs=1))
        pkp = ctx.enter_context(tc.tile_pool(name="pk", bufs=1))
        chp = ctx.enter_context(tc.tile_pool(name="ch", bufs=6))
        ch2p = ctx.enter_context(tc.tile_pool(name="ch2", bufs=2))
        vspp = ctx.enter_context(tc.tile_pool(name="vs", bufs=2))
        rsvp = ctx.enter_context(tc.tile_pool(name="rs", bufs=2))
        mkp = ctx.enter_context(tc.tile_pool(name="mk", bufs=2))
        ixp = ctx.enter_context(tc.tile_pool(name="ix", bufs=2))
        psp = ctx.enter_context(tc.tile_pool(name="ps", bufs=1, space="PSUM"))

        def bcast(dram_ap, n, name):
            t = consts.tile([128, n], F32, name=name)
            nc.sync.dma_start(t[:], dram_ap.partition_broadcast(128))
            return t

        def rsc(t, name, op=ALU.add):
            out = consts.tile([128, 1], F32, name=name)
            nc.vector.tensor_reduce(out[:], t[:], AX.X, op)
            return out

        def tmul(a, b, name, pool=None):
            out = (pool or smalls).tile(list(a.shape), F32, name=name)
            nc.vector.tensor_tensor(out=out[:], in0=a[:], in1=b[:],
                                    op=ALU.mult)
            return out

        # ------------- constants -------------
        wb = bcast(W1.ap(), 20, "wb")
        a1s = bcast(a_src1.ap(), 20, "a1s")
        a1d = bcast(a_dst1.ap(), 20, "a1d")
        a2s = bcast(a_src2.ap(), 20, "a2s")
        a2d = bcast(a_dst2.ap(), 20, "a2d")
        b2t = bcast(b2.ap(), 20, "b2t")
        wlt = bcast(Wl.ap(), 20, "wlt")
        blt = bcast(bl.ap(), 1, "blt")
        w2t = bcast(W2T.ap(), 400, "w2t")

        c1 = rsc(tmul(wb, a1s, "c1m"), "c1")
        c2 = rsc(tmul(wb, a1d, "c2m"), "c2")
        c12 = consts.tile([128, 1], F32, name="c12")
        nc.vector.tensor_tensor(out=c12[:], in0=c1[:], in1=c2[:], op=ALU.add)

        wp_ = consts.tile([128, 20], F32, name="wp_")
        nc.scalar.activation(wp_[:], wb[:], AF.Relu)
        wm_ = consts.tile([128, 20], F32, name="wm_")
        nc.scalar.activation(wm_[:], wb[:], AF.Relu, scale=-1.0)

        def qvec(wv, name):
            t = smalls.tile([128, 400], F32, name=name + "_t", tag="q400")
            nc.vector.tensor_tensor(
                out=t[:], in0=w2t[:],
                in1=wv[:].unsqueeze(1).broadcast_to([128, 20, 20]),
                op=ALU.mult)
            out = consts.tile([128, 20], F32, name=name)
            nc.vector.tensor_reduce(
                out[:], t[:].rearrange("p (j k) -> p j k", j=20), AX.X,
                ALU.add)
            return out

        qp = qvec(wp_, "qp")
        qm = qvec(wm_, "qm")
        A2 = rsc(tmul(qp, a2s, "A2m"), "A2")
        B2 = rsc(tmul(qm, a2s, "B2m"), "B2")
        C2 = rsc(tmul(qp, a2d, "C2m"), "C2")
        D2 = rsc(tmul(qm, a2d, "D2m"), "D2")
        AC = consts.tile([128, 1], F32, name="AC")
        nc.vector.tensor_tensor(out=AC[:], in0=A2[:], in1=C2[:], op=ALU.add)
        BD = consts.tile([128, 1], F32, name="BD")
        nc.vector.tensor_tensor(out=BD[:], in0=B2[:], in1=D2[:], op=ALU.add)

        selm_t = consts.tile([128, 16], F32, name="selm_t")
        nc.sync.dma_start(selm_t[:], selm_d.ap())
        ends_t = consts.tile([128, NSHP // 16], I16, name="ends_t")
        nc.sync.dma_start(ends_t[:], ends_d.ap())

        def nt(name):
            return nodep.tile([128, W98], F32, name=name)

        def dump_dbg(nm, t):
            if nm in dbg_outs:
                nc.sync.dma_start(dbg_outs[nm].ap(), t[:])

        # ------------- node inputs -------------
        xnode = nt("xnode")
        nc.sync.dma_start(xnode[:], xsh.ap())

        # b1 = exp(c2 * x_shard)  -> b1_dram
        t98 = nt("t98")
        nc.vector.tensor_scalar(out=t98[:], in0=xnode[:], scalar1=c2[:],
                                scalar2=None, op0=ALU.mult)
        b1n = nt("b1n")
        nc.scalar.activation(b1n[:], t98[:], AF.Exp)
        nc.sync.dma_start(b1_dram.ap(), b1n[:])

        # ------------- shared edge-layer routine -------------
        def edge_layer(li, tab):
            d = 2 if li == 1 else 3
            IV = bigp.tile([128, L * d], F32, tag="big", name=f"IV{li}")
            packed = pkp.tile([128, L], F32, tag="pk", name=f"packed{li}")
            for p in range(16):
                it = ixp.tile([128, LW], I16, tag="ix", name=f"ix{li}_{p}")
                nc.sync.dma_start(it[:], idxw_d.ap()[128 * p:128 * (p + 1), :])
                gt = gxp.tile([128, L], F32, tag="gx", name=f"gt{li}_{p}")
                nc.gpsimd.ap_gather(gt[:], tab[:], it[:], channels=128,
                                    num_elems=TABW, d=1, num_idxs=L)
                for b in range(8):
                    eng = (nc.sync, nc.scalar)[b % 2]
                    eng.dma_start(packed[16 * b + p:16 * b + p + 1, :],
                                  gt[16 * b:16 * b + 1, :])

            IVv = IV[:].rearrange("p (l dd) -> p l dd", dd=d)
            vlast = 0.0
            slast = [0.0] * d
            for q in range(NQ):
                sl = slice(q * LQ, (q + 1) * LQ)
                dm = mkp.tile([128, LQ], BF16, tag="mka", name=f"dm{li}_{q}")
                nc.sync.dma_start(dm[:], dmask_d.ap()[:, sl])
                km = mkp.tile([128, LQ], BF16, tag="mkb", name=f"km{li}_{q}")
                nc.scalar.dma_start(km[:], kill_d.ap()[:, sl])
                g = packed[:, sl]

                def CH(name, pool=chp):
                    return pool.tile([128, LQ], F32, tag="ch",
                                     name=f"{name}{li}_{q}")

                gd = CH("gd")
                nc.vector.tensor_tensor(out=gd[:], in0=g, in1=dm[:],
                                        op=ALU.mult)
                rsv = rsvp.tile([128, LQ], BF16, tag="rs",
                                name=f"rsv{li}_{q}")
                nc.vector.tensor_scalar(out=rsv[:], in0=dm[:], scalar1=-1.0,
                                        scalar2=1.0, op0=ALU.mult, op1=ALU.add)
                vsp = vspp.tile([128, LQ], F32, tag="vs", name=f"vsp{li}_{q}")
                nc.vector.tensor_tensor_scan(vsp[:], rsv[:], gd[:], vlast,
                                             ALU.mult, ALU.add)
                vlast = vsp[:, LQ - 1:LQ]

                if li == 1:
                    tk = CH("tk")
                    nc.vector.scalar_tensor_tensor(out=tk[:], in0=g,
                                                   scalar=c1[:], in1=km[:],
                                                   op0=ALU.mult, op1=ALU.mult)
                    a = CH("a")
                    nc.scalar.activation(a[:], tk[:], AF.Exp)
                    extras = [g]
                else:
                    gk = CH("gk")
                    nc.vector.tensor_tensor(out=gk[:], in0=g, in1=km[:],
                                            op=ALU.mult)
                    rpe = CH("rpe", ch2p)
                    nc.scalar.activation(rpe[:], gk[:], AF.Relu)
                    rme = CH("rme", ch2p)
                    nc.scalar.activation(rme[:], gk[:], AF.Relu, scale=-1.0)
                    u2 = CH("u2")
                    nc.vector.tensor_scalar(out=u2[:], in0=rpe[:],
                                            scalar1=A2[:], scalar2=None,
                                            op0=ALU.mult)
                    u2b = CH("u2b")
                    nc.vector.scalar_tensor_tensor(out=u2b[:], in0=rme[:],
                                                   scalar=B2[:], in1=u2[:],
                                                   op0=ALU.mult, op1=ALU.add)
                    a = CH("a")
                    nc.scalar.activation(a[:], u2b[:], AF.Exp)
                    extras = [rpe, rme]

                w = CH("w")
                nc.vector.tensor_tensor(out=w[:], in0=a[:], in1=vsp[:],
                                        op=ALU.mult)
                wc = CH("wc")
                nc.vector.tensor_scalar(out=wc[:], in0=w[:], scalar1=1e-35,
                                        scalar2=None, op0=ALU.max)
                lnw = CH("lnw")
                nc.scalar.activation(lnw[:], wc[:], AF.Ln)
                wpow = CH("wpow")
                nc.scalar.activation(wpow[:], lnw[:], AF.Exp, scale=0.2)
                nm = CH("nm")
                nc.vector.tensor_tensor(out=nm[:], in0=wc[:], in1=wpow[:],
                                        op=ALU.max)
                nmf = CH("nmf")
                nc.vector.tensor_tensor(out=nmf[:], in0=nm[:], in1=km[:],
                                        op=ALU.mult)

                streams = [nmf]
                for ci, ext in enumerate(extras):
                    t = CH(f"wch{ci}")
                    ein = ext if isinstance(ext, bass.AP) else ext[:]
                    nc.vector.tensor_tensor(out=t[:], in0=nmf[:], in1=ein,
                                            op=ALU.mult)
                    streams.append(t)
                for ci, st in enumerate(streams):
                    out_ap = IVv[:, q * LQ:(q + 1) * LQ,
                                 ci:ci + 1].squeeze(-1)
                    nc.vector.tensor_tensor_scan(out_ap, rsv[:], st[:],
                                                 slast[ci], ALU.mult, ALU.add)
                    slast[ci] = IVv[:, (q + 1) * LQ - 1:(q + 1) * LQ,
                                    ci:ci + 1].squeeze(-1)

            # boundary extraction: piece-aligned chunks
            SD = S1 if li == 1 else S2
            CIDX = 1568
            NCH = NSHP // CIDX
            for cq in range(NCH):
                cd = gxp.tile([128, CIDX * d], F32, tag="gx",
                              name=f"cd{li}_{cq}")
                cw = CIDX // 16
                nc.gpsimd.ap_gather(
                    cd[:].rearrange("p (l dd) -> p l dd", dd=d), IVv,
                    ends_t[:, cw * cq:cw * (cq + 1)],
                    channels=128, num_elems=L, d=d, num_idxs=CIDX)
                nblk = CIDX // 784
                for blk in range(nblk):
                    piece = nblk * cq + blk
                    ps = psp.tile([1, 784 * d], F32, tag="ps",
                                  name=f"ps{li}_{cq}_{blk}")
                    base = blk * 784 * d
                    off = 0
                    while off < 784 * d:
                        kn = min(512, 784 * d - off)
                        nc.tensor.matmul(ps[:, off:off + kn],
                                         selm_t[:, piece:piece + 1],
                                         cd[:, base + off:base + off + kn],
                                         start=True, stop=True)
                        off += kn
                    ev = smalls.tile([1, 784 * d], F32, tag="ev",
                                     name=f"ev{li}_{cq}_{blk}", bufs=2)
                    nc.scalar.copy(ev[:], ps[:])
                    dst0 = (CIDX * cq + 784 * blk) * d
                    nc.sync.dma_start(SD.ap()[:, dst0:dst0 + 784 * d], ev[:])

            sums = nodep.tile([128, W98 * d], F32, name=f"sums{li}")
            nc.sync.dma_start(sums[:], SD.ap())
            return sums[:].rearrange("p (w dd) -> p w dd", dd=d)

        # ------------- layer 1 -------------
        tab1 = bigp.tile([128, TABW], F32, tag="big", name="tab1")
        for b in range(8):
            nc.sync.dma_start(tab1[16 * b:16 * b + 1, 0:XW],
                              xc.ap()[:, b * NSHP:b * NSHP + XW])
            nc.scalar.dma_start(tab1[16 * b:16 * b + 1, XW:XW + NSHP - 32],
                                b1_dram.ap()[:, 0:NSHP - 32])
        nc.vector.memset(tab1[:, ZCOL:TABW], 0.0)

        sv1 = edge_layer(1, tab1)

        # node phase L1: add self-loop terms, normalize
        zs = nt("zs")
        nc.vector.tensor_scalar(out=zs[:], in0=xnode[:], scalar1=c12[:],
                                scalar2=None, op0=ALU.mult)
        aes = nt("aes")
        nc.vector.scalar_tensor_tensor(out=aes[:], in0=zs[:], scalar=0.2,
                                       in1=zs[:], op0=ALU.mult, op1=ALU.max)
        ns1 = nt("ns1")
        nc.scalar.activation(ns1[:], aes[:], AF.Exp)
        den1 = nt("den1")
        nc.vector.tensor_tensor(out=den1[:], in0=sv1[:, :, 0:1], in1=ns1[:],
                                op=ALU.add)
        nx = nt("nx")
        nc.vector.tensor_tensor(out=nx[:], in0=ns1[:], in1=xnode[:],
                                op=ALU.mult)
        P1 = nt("P1")
        nc.vector.tensor_tensor(out=P1[:], in0=sv1[:, :, 1:2], in1=nx[:],
                                op=ALU.add)
        den1e = nt("den1e")
        nc.vector.tensor_scalar(out=den1e[:], in0=den1[:], scalar1=1e-30,
                                scalar2=None, op0=ALU.add)
        rec1 = nt("rec1")
        nc.vector.reciprocal(rec1[:], den1e[:])
        Pn = nt("Pn")
        nc.vector.tensor_tensor(out=Pn[:], in0=P1[:], in1=rec1[:],
                                op=ALU.mult)
        dump_dbg("den1", den1)
        dump_dbg("P1", P1)
        dump_dbg("Pn", Pn)

        nc.sync.dma_start(p_local.ap(), Pn[:])
        nc.gpsimd.collective_compute(
            "AllGather", ALU.bypass, replica_groups=[list(range(8))],
            ins=[p_local.ap()], outs=[p_full.ap()])

        # b2 = exp(C2*relu(Pn) + D2*relu(-Pn)) -> b2_dram
        rpn = nt("rpn")
        nc.scalar.activation(rpn[:], Pn[:], AF.Relu)
        rmn = nt("rmn")
        nc.scalar.activation(rmn[:], Pn[:], AF.Relu, scale=-1.0)
        v2a = nt("v2a")
        nc.vector.tensor_scalar(out=v2a[:], in0=rpn[:], scalar1=C2[:],
                                scalar2=None, op0=ALU.mult)
        v2n = nt("v2n")
        nc.vector.scalar_tensor_tensor(out=v2n[:], in0=rmn[:], scalar=D2[:],
                                       in1=v2a[:], op0=ALU.mult, op1=ALU.add)
        b2n = nt("b2n")
        nc.scalar.activation(b2n[:], v2n[:], AF.Exp)
        nc.sync.dma_start(b2_dram.ap(), b2n[:])

        # ------------- layer 2 -------------
        tab2 = bigp.tile([128, TABW], F32, tag="big", name="tab2")
        for b in range(8):
            nc.sync.dma_start(tab2[16 * b:16 * b + 1, 0:XW],
                              p_full.ap()[:, b * NSHP:b * NSHP + XW])
            nc.scalar.dma_start(tab2[16 * b:16 * b + 1, XW:XW + NSHP - 32],
                                b2_dram.ap()[:, 0:NSHP - 32])
        nc.vector.memset(tab2[:, ZCOL:TABW], 0.0)

        sv2 = edge_layer(2, tab2)

        if dbg:
            s20 = nt("s20")
            nc.vector.tensor_scalar(out=s20[:], in0=sv2[:, :, 0:1],
                                    scalar1=0.0, scalar2=None, op0=ALU.add)
            dump_dbg("s20", s20)
            s21 = nt("s21")
            nc.vector.tensor_scalar(out=s21[:], in0=sv2[:, :, 1:2],
                                    scalar1=0.0, scalar2=None, op0=ALU.add)
            dump_dbg("s21", s21)
            dump_dbg("b2nd", b2n)
            pfs = nt("pfs")
            nc.sync.dma_start(pfs[:], p_full.ap()[:, 0:NSHP])
            dump_dbg("pf", pfs)

        # node phase L2
        za = nt("za")
        nc.vector.tensor_scalar(out=za[:], in0=rpn[:], scalar1=AC[:],
                                scalar2=None, op0=ALU.mult)
        zb = nt("zb")
        nc.vector.scalar_tensor_tensor(out=zb[:], in0=rmn[:], scalar=BD[:],
                                       in1=za[:], op0=ALU.mult, op1=ALU.add)
        aes2 = nt("aes2")
        nc.vector.scalar_tensor_tensor(out=aes2[:], in0=zb[:], scalar=0.2,
                                       in1=zb[:], op0=ALU.mult, op1=ALU.max)
        ns2 = nt("ns2")
        nc.scalar.activation(ns2[:], aes2[:], AF.Exp)
        den2 = nt("den2")
        nc.vector.tensor_tensor(out=den2[:], in0=sv2[:, :, 0:1], in1=ns2[:],
                                op=ALU.add)
        nsp = nt("nsp")
        nc.vector.tensor_tensor(out=nsp[:], in0=ns2[:], in1=rpn[:],
                                op=ALU.mult)
        Sp = nt("Sp")
        nc.vector.tensor_tensor(out=Sp[:], in0=sv2[:, :, 1:2], in1=nsp[:],
                                op=ALU.add)
        nsm = nt("nsm")
        nc.vector.tensor_tensor(out=nsm[:], in0=ns2[:], in1=rmn[:],
                                op=ALU.mult)
        Sm = nt("Sm")
        nc.vector.tensor_tensor(out=Sm[:], in0=sv2[:, :, 2:3], in1=nsm[:],
                                op=ALU.add)
        den2e = nt("den2e")
        nc.vector.tensor_scalar(out=den2e[:], in0=den2[:], scalar1=1e-30,
                                scalar2=None, op0=ALU.add)
        rec2 = nt("rec2")
        nc.vector.reciprocal(rec2[:], den2e[:])
        Rp = nt("Rp")
        nc.vector.tensor_tensor(out=Rp[:], in0=Sp[:], in1=rec2[:],
                                op=ALU.mult)
        Rm = nt("Rm")
        nc.vector.tensor_tensor(out=Rm[:], in0=Sm[:], in1=rec2[:],
                                op=ALU.mult)
        dump_dbg("den2", den2)
        dump_dbg("Rp", Rp)
        dump_dbg("Rm", Rm)

        # y[d] = bl + sum_k relu(Rp*qp_k + Rm*qm_k + b2_k) * Wl_k
        yacc = nt("yacc")
        nc.vector.memset(yacc[:], 0.0)
        for k in range(20):
            tk_ = smalls.tile([128, W98], F32, name=f"yk{k}", tag="yk", bufs=3)
            nc.vector.tensor_scalar(out=tk_[:], in0=Rp[:],
                                    scalar1=qp[:, k:k + 1], scalar2=None,
                                    op0=ALU.mult)
            nc.vector.scalar_tensor_tensor(out=tk_[:], in0=Rm[:],
                                           scalar=qm[:, k:k + 1], in1=tk_[:],
                                           op0=ALU.mult, op1=ALU.add)
            hk = smalls.tile([128, W98], F32, name=f"hk{k}", tag="yk", bufs=3)
            nc.scalar.activation(hk[:], tk_[:], AF.Relu,
                                 bias=b2t[:, k:k + 1])
            nc.vector.scalar_tensor_tensor(out=yacc[:], in0=hk[:],
                                           scalar=wlt[:, k:k + 1],
                                           in1=yacc[:], op0=ALU.mult,
                                           op1=ALU.add)
        yf = nt("yf")
        nc.vector.tensor_scalar(out=yf[:], in0=yacc[:], scalar1=blt[:],
                                scalar2=None, op0=ALU.add)
        nc.sync.dma_start(y_out.ap(), yf[:])

    nc.compile()
    return nc


def make_in_maps(pp, inputs):
    L, LW = pp["L"], pp["LW"]
    x = np.asarray(inputs["x"], np.float32).reshape(-1)
    xc = np.zeros(NC * NSHP, np.float32)
    for b in range(NC):
        xc[b * NSHP:b * NSHP + NSH] = x[b * NSH:(b + 1) * NSH]
    W2T = np.ascontiguousarray(np.asarray(inputs["W2"], np.float32).T)

    common = {
        "xc": xc[None, :],
        "W1": np.asarray(inputs["W1"], np.float32).reshape(1, 20),
        "a_src1": np.asarray(inputs["a_src1"], np.float32).reshape(1, 20),
        "a_dst1": np.asarray(inputs["a_dst1"], np.float32).reshape(1, 20),
        "W2T": W2T.reshape(1, 400),
        "a_src2": np.asarray(inputs["a_src2"], np.float32).reshape(1, 20),
        "a_dst2": np.asarray(inputs["a_dst2"], np.float32).reshape(1, 20),
        "b2": np.asarray(inputs["b2"], np.float32).reshape(1, 20),
        "Wl": np.asarray(inputs["Wl"], np.float32).reshape(1, 20),
        "bl": np.asarray(inputs["bl"], np.float32).reshape(1, 1),
    }
    maps = []
    for c in range(NC):
        pc = pp["cores"][c]
        maps.append({
            **common,
            "xsh": xc[c * NSHP:(c + 1) * NSHP][None, :],
            "idxw": pc["idxw"],
            "dmask": pc["dmask"],
            "kill": pc["kill"],
            "ends_w": pc["ends_w"],
            "selm": pc["selm"],
        })
    return maps


# ===================== runner =====================

def _run_spmd(nc, maps):
    from concourse.bass_utils import run_bass_kernel_spmd
    return run_bass_kernel_spmd(nc, maps, list(range(8)))


def _host_guard_ok(inputs):
    x = np.asarray(inputs["x"], np.float32).reshape(-1)
    xmax = float(np.abs(x).max())
    W1 = np.asarray(inputs["W1"], np.float32)
    c1 = float(W1[0] @ np.asarray(inputs["a_src1"], np.float32))
    c2 = float(W1[0] @ np.asarray(inputs["a_dst1"], np.float32))
    W2 = np.asarray(inputs["W2"], np.float32)
    qp = np.maximum(W1, 0.0)[0] @ W2
    qm = np.maximum(-W1, 0.0)[0] @ W2
    A2 = float(qp @ np.asarray(inputs["a_src2"], np.float32))
    B2 = float(qm @ np.asarray(inputs["a_src2"], np.float32))
    C2 = float(qp @ np.asarray(inputs["a_dst2"], np.float32))
    D2 = float(qm @ np.asarray(inputs["a_dst2"], np.float32))
    ub1 = (abs(c1) + abs(c2)) * xmax
    ub2 = (abs(A2) + abs(B2) + abs(C2) + abs(D2)) * xmax
    return max(ub1, ub2) < 45.0


def kernel(**inputs):
    x = np.asarray(inputs["x"], np.float32)
    N = x.shape[0]
    if (N != N_NODES or np.any(np.asarray(inputs["b1"]))
            or not _host_guard_ok(inputs)):
        return _kernel_numpy(**inputs)
    pp = prep(np.asarray(inputs["edge_index"]), N)
    nc = build(pp, dbg=False)
    maps = make_in_maps(pp, inputs)
    res = _run_spmd(nc, maps)
    y = np.zeros((N, 1), np.float32)
    for c in range(NC):
        y[c * NSH:(c + 1) * NSH, 0] = res.results[c]["y"].reshape(-1)[:NSH]
    return y


def _kernel_numpy(x, edge_index, W1, a_src1, a_dst1, b1, W2, a_src2, a_dst2,
                  b2, Wl, bl):
    def lr(v):
        return np.where(v > 0, v, 0.2 * v).astype(np.float32)

    def conv(h, src, dst, W, asrc, adst, b, n):
        hh = (h @ W).astype(np.float32)
        u, v = hh @ asrc, hh @ adst
        e = lr(u[src] + v[dst])
        m = np.full(n, -np.inf, np.float32)
        np.maximum.at(m, dst, e)
        ee = np.exp(e - m[dst]).astype(np.float32)
        den = np.bincount(dst, weights=ee, minlength=n).astype(np.float32)
        al = ee / (den[dst] + 1e-16)
        out = np.zeros((n, hh.shape[1]), np.float32)
        wh = hh[src] * al[:, None]
        for k in range(hh.shape[1]):
            out[:, k] = np.bincount(dst, weights=wh[:, k], minlength=n)
        return out + b

    n = x.shape[0]
    loop = np.arange(n, dtype=np.int64)
    src = np.concatenate([np.asarray(edge_index[0]), loop])
    dst = np.concatenate([np.asarray(edge_index[1]), loop])
    h = np.maximum(conv(np.asarray(x, np.float32), src, dst, W1, a_src1,
                        a_dst1, b1, n), 0)
    h = np.maximum(conv(h, src, dst, W2, a_src2, a_dst2, b2, n), 0)
    return (h @ Wl + bl).astype(np.float32)
